# revision 9
# baseline (speedup 1.0000x reference)
"""DCM (dynamic conv module) Trainium2 kernel, v2.

Reference computation (per sample b, channel c):
  f[b,c,3,3]  = adaptive_avg_pool2d(x[b,c], 3)        # dynamic depthwise filter
  out[b,c]    = depthwise_conv3x3(x[b,c], f[b,c])     # zero padding 1
  y           = relu(batchnorm_train(out, gamma, beta))  # batch stats over (B,H,W)

Sharding: data-parallel over batch B=16 across 8 cores (2 samples/core).
Sync-BN via a [C,2] AllReduce of per-channel (sum, sumsq).

v2 design (vs v1 which ran all 9 taps as diag matmuls on PE):
  - x is shipped fp16 (host cast) and held resident in SBUF in a padded
    [C, (H+2), (W+1)] layout per plane: zero top/bottom rows plus a zero
    column appended to each row, so every one of the 9 shifted tap reads
    lands on zeros at the borders. x streams from HBM exactly once and no
    edge fixups are needed anywhere.
  - The 9 taps are spread across all four compute engines:
      ACT    1   tap  (activation Copy w/ per-channel scale, writes PSUM)
      PE     3.5 taps (fp16 diag matmuls accumulate onto ACT's write,
                       start=False; the .5 alternates with DVE by tile)
      DVE    2.5 taps (scalar_tensor_tensor RMW on PSUM; the final tap
                       merges PSUM into the fp16 result tile)
      GPSIMD 2   taps (stt RMW on the fp16 result in SBUF; the last one
                       also emits the per-channel sum via accum_out)
  - Pooling runs on ACT as 9 rectangle sums (activation Copy + accum_out)
    per plane; sumsq on ACT (Square + accum_out). Copy/Square/Relu share
    one activation table -> no table reloads.
  - BN apply is DVE-only in fp16 4x mode (tensor_scalar mult/add + max),
    output DMA'd as fp16; the host casts back to f32.
"""

import numpy as np

# ---------------------------------------------------------------- constants
B, C, H, W = 16, 128, 128, 128
N_CORES = 8
BL = B // N_CORES          # samples per core
HW = H * W
FS = 3
BN_EPS = 1e-5

RW = W + 1                 # padded row stride (extra zero column)
XROWS = H + 2              # padded rows (zero row top + bottom)
XF = XROWS * RW + 2        # fp16 elems per plane (+1 zero pad each end)

TROWS = 8                  # output rows per psum tile
NT = H // TROWS            # tiles per plane
TF = TROWS * W             # 1024 free elems per psum tile
NTILE = NT * BL            # psum tiles per core (32)
MM_N = 512                 # max moving free per matmul (psum bank limit)

# adaptive_avg_pool2d(3) bin boundaries (PyTorch convention)
SH = [(i * H) // FS for i in range(FS)]
EH = [-((-(i + 1) * H) // FS) for i in range(FS)]
SW = [(i * W) // FS for i in range(FS)]
EW = [-((-(i + 1) * W) // FS) for i in range(FS)]

# taps t = 3*(di+1)+(dj+1), (di,dj) in {-1,0,1}^2
TAPS = [(di, dj) for di in (-1, 0, 1) for dj in (-1, 0, 1)]
T_ACT = 4                  # (0,0)   ACT chain start (writes psum)
T_PE = [0, 1, 2, 3]        # PE diag-matmul taps (accumulate onto psum)
T_DVE_FIN = 5              # (0,1)   DVE stt: res = x*f + psum (fp16 out)
T_TS_GP = 6                # (1,-1)  DVE ts 4x scale, GPSIMD TT accumulate
T_TS_DVE = 7               # (1,0)   DVE ts 4x scale, DVE TT 2x accumulate
T_DVE_LAST = 8             # (1,1)   DVE stt RMW on res, accum_out -> sums


def _counts_recip():
    cr = np.empty((C, FS * FS), dtype=np.float32)
    for i in range(FS):
        for j in range(FS):
            cr[:, 3 * i + j] = 1.0 / float((EH[i] - SH[i]) * (EW[j] - SW[j]))
    return cr


def build_nc(n_cores: int = N_CORES):
    """Build + compile the per-core Bass program (identical on all cores)."""
    import concourse.bacc as bacc
    import concourse.tile as tile
    from concourse import mybir

    f32 = mybir.dt.float32
    f16 = mybir.dt.float16
    AT = mybir.ActivationFunctionType
    OP = mybir.AluOpType

    ntot = float(n_cores * BL * HW)   # BN element count per channel

    nc = bacc.Bacc(
        "TRN2",
        target_bir_lowering=False,
        debug=False,
        num_devices=n_cores,
    )

    x_d = nc.dram_tensor("x", [BL, C, HW], f16, kind="ExternalInput").ap()
    gamma_d = nc.dram_tensor("gamma", [C, 1], f32, kind="ExternalInput").ap()
    beta_d = nc.dram_tensor("beta", [C, 1], f32, kind="ExternalInput").ap()
    ident_d = nc.dram_tensor("ident", [C, C], f16, kind="ExternalInput").ap()
    crecip_d = nc.dram_tensor("crecip", [C, FS * FS], f32, kind="ExternalInput").ap()
    y_d = nc.dram_tensor("y", [BL, C, HW], f16, kind="ExternalOutput").ap()

    with tile.TileContext(nc) as tc:
        with (
            tc.tile_pool(name="singles", bufs=1) as singles,
            tc.tile_pool(name="xres", bufs=BL) as xresp,
            tc.tile_pool(name="res", bufs=BL) as resp,
            tc.tile_pool(name="psum", bufs=4, space="PSUM") as psum,
            tc.tile_pool(name="scr", bufs=2) as scrp,
            tc.tile_pool(name="sq", bufs=2) as sqp,
            tc.tile_pool(name="fpool", bufs=2) as fpool,
            tc.tile_pool(name="diagp", bufs=2 * FS * FS) as diagp,
            tc.tile_pool(name="statp", bufs=1) as statp,
            tc.tile_pool(name="dram", bufs=1, space="DRAM") as dram,
        ):
            # ---- constants
            gamma_s = singles.tile([C, 1], f32, tag="gamma")
            nc.sync.dma_start(out=gamma_s[:], in_=gamma_d[:, :])
            beta_s = singles.tile([C, 1], f32, tag="beta")
            nc.sync.dma_start(out=beta_s[:], in_=beta_d[:, :])
            ident_s = singles.tile([C, C], f16, tag="ident")
            nc.sync.dma_start(out=ident_s[:], in_=ident_d[:, :])
            crecip_s = singles.tile([C, FS * FS], f32, tag="crecip")
            nc.sync.dma_start(out=crecip_s[:], in_=crecip_d[:, :])

            sums = statp.tile([C, NTILE], f32, tag="sums")
            sumsq = statp.tile([C, NTILE], f32, tag="sumsq")

            # Dummy warm-up AllReduce issued at kernel start: absorbs the
            # one-time ncfw ramp so the real stats AllReduce is cheaper.
            warm = statp.tile([C, 2], f32, tag="warm")
            nc.gpsimd.memset(warm[:], 0.0)
            dw_in = dram.tile([C, 2], f32, tag="dw_in")
            dw_out = dram.tile([C, 2], f32, tag="dw_out")
            nc.sync.dma_start(out=dw_in[:], in_=warm[:])
            nc.gpsimd.collective_compute(
                "AllReduce",
                OP.add,
                replica_groups=[list(range(n_cores))],
                ins=[dw_in[:].opt()],
                outs=[dw_out[:].opt()],
            )

            # ---- resident x planes (padded) + result planes
            xts, xvs, rts = [], [], []
            for s in range(BL):
                xt = xresp.tile([C, XF], f16, tag="xres")
                xv = xt[:, 1:1 + XROWS * RW].rearrange("p (r w) -> p r w", w=RW)
                # zero pads: 1-elem ends, top row, bottom row, extra column
                nc.vector.memset(xt[:, 0:1], 0.0)
                nc.vector.memset(xt[:, XF - 1:XF], 0.0)
                nc.vector.memset(xv[:, 0:1, :], 0.0)
                nc.vector.memset(xv[:, XROWS - 1:XROWS, :], 0.0)
                nc.vector.memset(xv[:, :, W:RW], 0.0)
                xts.append(xt)
                xvs.append(xv)
                rt = resp.tile([C, HW], f16, tag="res")
                rts.append(rt)

            # band-wise plane loads so pooling can start before a full plane
            # lands; bands match the adaptive-pool row bins
            for s in range(BL):
                for i in range(FS):
                    r0, r1 = SH[i], EH[i]
                    nc.sync.dma_start(
                        out=xvs[s][:, 1 + r0:1 + r1, 0:W],
                        in_=x_d[s, :, r0 * W:r1 * W].rearrange(
                            "p (r w) -> p r w", w=W
                        ),
                    )

            # ---------------- per-plane pooling (ACT) + diag weights
            fTs, diags = [], []
            for s in range(BL):
                rect = fpool.tile([C, FS * FS], f32, tag="rect")
                scr = scrp.tile([C, 44 * 44], f16, tag="scr")
                for i in range(FS):
                    bh = EH[i] - SH[i]
                    for j in range(FS):
                        bw = EW[j] - SW[j]
                        sv = scr[:, 0:bh * bw].rearrange(
                            "p (r w) -> p r w", w=bw
                        )
                        nc.scalar.activation(
                            out=sv,
                            in_=xvs[s][:, 1 + SH[i]:1 + EH[i], SW[j]:EW[j]],
                            func=AT.Copy,
                            accum_out=rect[:, 3 * i + j:3 * i + j + 1],
                        )
                fT = fpool.tile([C, FS * FS], f32, tag="fT")
                nc.vector.tensor_mul(fT[:], rect[:], crecip_s[:])
                fTs.append(fT)
                dgs = {}
                for t in T_PE:
                    dg = diagp.tile([C, C], f16, tag="diag")
                    nc.vector.tensor_scalar_mul(dg[:], ident_s[:], fT[:, t:t + 1])
                    dgs[t] = dg
                diags.append(dgs)

            # ---------------- conv: 9 taps spread across engines
            def tapview(s, r0, t, rows=TROWS):
                """[C, rows, W] view of x shifted by tap t for out rows r0.."""
                di, dj = TAPS[t]
                ofs = 1 + (r0 + di + 1) * RW + dj
                return xts[s][:, ofs:ofs + rows * RW].rearrange(
                    "p (r w) -> p r w", w=RW
                )[:, :, 0:W]

            for k in range(NTILE):
                s, i = divmod(k, NT)
                r0 = i * TROWS
                fT = fTs[s]
                pt = psum.tile([C, TF], f32, tag="pt")
                ptv = pt[:].rearrange("p (r w) -> p r w", w=W)

                # ACT writes tap T_ACT into psum (chain start)
                nc.scalar.activation(
                    out=ptv, in_=tapview(s, r0, T_ACT), func=AT.Copy,
                    scale=fT[:, T_ACT:T_ACT + 1],
                )
                # PE accumulates its taps on top (start=False)
                for n, t in enumerate(T_PE):
                    last = n == len(T_PE) - 1
                    for h in range(2):
                        nc.tensor.matmul(
                            pt[:, h * MM_N:(h + 1) * MM_N],
                            diags[s][t][:],
                            tapview(s, r0, t)[:, h * 4:(h + 1) * 4, :],
                            start=False,
                            stop=last and h == 1,
                        )
                # DVE stt merges psum + tap -> fp16 res
                rv = rts[s][:, r0 * W:(r0 + TROWS) * W].rearrange(
                    "p (r w) -> p r w", w=W
                )
                t = T_DVE_FIN
                nc.vector.scalar_tensor_tensor(
                    out=rv, in0=tapview(s, r0, t),
                    scalar=fT[:, t:t + 1], in1=ptv,
                    op0=OP.mult, op1=OP.add,
                )
                # tap T_TS_GP: DVE 4x scale into scratch, GPSIMD TT accumulate
                t = T_TS_GP
                tmp6 = sqp.tile([C, TF], f16, tag="tmp6")
                tv6 = tmp6[:].rearrange("p (r w) -> p r w", w=W)
                nc.vector.tensor_scalar_mul(tv6, tapview(s, r0, t), fT[:, t:t + 1])
                nc.gpsimd.tensor_add(rv, rv, tv6)
                # tap T_TS_DVE: DVE 4x scale + DVE 2x TT accumulate
                t = T_TS_DVE
                tmp7 = sqp.tile([C, TF], f16, tag="tmp7")
                tv7 = tmp7[:].rearrange("p (r w) -> p r w", w=W)
                nc.vector.tensor_scalar_mul(tv7, tapview(s, r0, t), fT[:, t:t + 1])
                nc.vector.tensor_add(rv, rv, tv7)
                # last tap: DVE stt RMW on res, accum emits the BN sum
                t = T_DVE_LAST
                nc.vector.scalar_tensor_tensor(
                    out=rv, in0=tapview(s, r0, t),
                    scalar=fT[:, t:t + 1], in1=rv,
                    op0=OP.mult, op1=OP.add,
                    accum_out=sums[:, k:k + 1],
                )
                # ACT sumsq (Square + accum)
                sq = sqp.tile([C, TF], f16, tag="sq")
                nc.scalar.activation(
                    out=sq[:], in_=rts[s][:, r0 * W:(r0 + TROWS) * W],
                    func=AT.Square,
                    accum_out=sumsq[:, k:k + 1],
                )

            # ---------------- sync-BN stats AllReduce
            arin = statp.tile([C, 2], f32, tag="arin")
            AX = mybir.AxisListType
            nc.vector.tensor_reduce(out=arin[:, 0:1], in_=sums[:], axis=AX.X, op=OP.add)
            nc.vector.tensor_reduce(out=arin[:, 1:2], in_=sumsq[:], axis=AX.X, op=OP.add)
            d_in = dram.tile([C, 2], f32, tag="d_in")
            d_out = dram.tile([C, 2], f32, tag="d_out")
            nc.sync.dma_start(out=d_in[:], in_=arin[:])
            nc.gpsimd.collective_compute(
                "AllReduce",
                OP.add,
                replica_groups=[list(range(n_cores))],
                ins=[d_in[:].opt()],
                outs=[d_out[:].opt()],
            )
            aro = statp.tile([C, 2], f32, tag="aro")
            nc.sync.dma_start(out=aro[:], in_=d_out[:])

            # ---------------- BN scale/shift (all [C,1], fp32)
            mean = statp.tile([C, 1], f32, tag="mean")
            nc.vector.tensor_scalar_mul(mean[:], aro[:, 0:1], 1.0 / ntot)
            ex2 = statp.tile([C, 1], f32, tag="ex2")
            nc.vector.tensor_scalar_mul(ex2[:], aro[:, 1:2], 1.0 / ntot)
            var = statp.tile([C, 1], f32, tag="var")
            nc.vector.tensor_mul(var[:], mean[:], mean[:])
            nc.vector.tensor_sub(var[:], ex2[:], var[:])
            veps = statp.tile([C, 1], f32, tag="veps")
            nc.vector.tensor_scalar_add(veps[:], var[:], BN_EPS)
            eps_t = statp.tile([C, 1], f32, tag="eps_t")
            nc.vector.memset(eps_t[:], BN_EPS)
            sd = statp.tile([C, 1], f32, tag="sd")
            nc.scalar.activation(out=sd[:], in_=var[:], func=AT.Sqrt, bias=eps_t[:])
            z = statp.tile([C, 1], f32, tag="z")
            nc.vector.reciprocal(z[:], sd[:])
            # one Newton step: z <- z * (1.5 - 0.5 * veps * z^2)
            nt = statp.tile([C, 1], f32, tag="nt")
            nc.vector.tensor_mul(nt[:], z[:], z[:])
            nc.vector.tensor_mul(nt[:], nt[:], veps[:])
            nc.vector.tensor_scalar(
                out=nt[:], in0=nt[:], scalar1=-0.5, scalar2=1.5,
                op0=OP.mult, op1=OP.add,
            )
            nc.vector.tensor_mul(z[:], z[:], nt[:])
            scale_t = statp.tile([C, 1], f32, tag="scale_t")
            nc.vector.tensor_mul(scale_t[:], gamma_s[:], z[:])
            shift_t = statp.tile([C, 1], f32, tag="shift_t")
            nc.vector.tensor_mul(shift_t[:], mean[:], scale_t[:])
            nc.vector.tensor_sub(shift_t[:], beta_s[:], shift_t[:])

            # ---------------- BN apply + ReLU (DVE fp16 4x) + writeback
            NCH = 4                      # chunks per plane
            CF = HW // NCH               # 4096
            for s in range(BL):
                for c in range(NCH):
                    cv = rts[s][:, c * CF:(c + 1) * CF]
                    nc.vector.tensor_scalar(
                        out=cv, in0=cv,
                        scalar1=scale_t[:], scalar2=shift_t[:],
                        op0=OP.mult, op1=OP.add,
                    )
                    nc.vector.tensor_scalar_max(cv, cv, 0.0)
                    nc.sync.dma_start(
                        out=y_d[s, :, c * CF:(c + 1) * CF], in_=cv,
                    )

    nc.compile()
    return nc


_NC_CACHE = {}


def _get_nc(n_cores: int = N_CORES):
    if n_cores not in _NC_CACHE:
        _NC_CACHE[n_cores] = build_nc(n_cores)
    return _NC_CACHE[n_cores]


def make_in_maps(x: np.ndarray, gamma: np.ndarray, beta: np.ndarray,
                 n_cores: int = N_CORES):
    x_r = np.ascontiguousarray(
        np.asarray(x, dtype=np.float32).reshape(B, C, HW).astype(np.float16)
    )
    g = np.ascontiguousarray(np.asarray(gamma, dtype=np.float32).reshape(C, 1))
    b = np.ascontiguousarray(np.asarray(beta, dtype=np.float32).reshape(C, 1))
    ident = np.eye(C, dtype=np.float16)
    crecip = _counts_recip()
    maps = []
    for core in range(n_cores):
        maps.append({
            "x": x_r[core * BL:(core + 1) * BL],
            "gamma": g,
            "beta": b,
            "ident": ident,
            "crecip": crecip,
        })
    return maps


def kernel(x, gamma, beta):
    from concourse import bass_utils

    nc = _get_nc(N_CORES)
    in_maps = make_in_maps(x, gamma, beta, N_CORES)
    res = bass_utils.run_bass_kernel_spmd(nc, in_maps, core_ids=list(range(N_CORES)))
    y = np.concatenate([res.results[c]["y"] for c in range(N_CORES)], axis=0)
    return y.reshape(B, C, H, W).astype(np.float32)
